# revision 59
# baseline (speedup 1.0000x reference)
"""Multi-head causal attention on 8 trn2 NeuronCores.

Sharding: data-parallel over batch (2) x tensor-parallel over heads (4 per
core, Megatron-style column-split QKV / row-split output projection).
Per-core partial outputs are summed on the host (+ output bias).

Schedule: the two head-pairs' attention streams are interleaved step by
step so every engine always has independent work between the dependent
score -> exp -> mask -> ctx hops of a single stream; q/k/v projection units
and the final-linear units ride along as fillers in the step slots.
"""

import sys

sys.path.insert(0, "/opt/trn_rl_repo")

import ml_dtypes
import numpy as np

import concourse.bass as bass  # noqa: F401  (import keeps bass registered)
import concourse.tile as tile
from concourse import bacc, mybir

BF16 = mybir.dt.bfloat16
F32 = mybir.dt.float32
AF = mybir.ActivationFunctionType

N = 2048  # sequence length
D = 1024  # model dim
NC = 8  # cores


def build_nc(variant="full", loop=1, unroll=1):
    """Build the (SPMD) Bass program run identically on all 8 cores.

    loop: repeat the whole body N times inside the NEFF (timing harness).
    unroll: bodies per For_i iteration — the loop's all-engine barrier
        (semaphore reset) fires once per `unroll` bodies, letting adjacent
        bodies pipeline (next body's DMAs/proj overlap this body's tail).
    """
    nc = bacc.Bacc("TRN2", target_bir_lowering=False, debug=False, num_devices=NC)

    xT = nc.declare_dram_parameter("xT", [8, 128, N], BF16, isOutput=False)
    wq = nc.declare_dram_parameter("wq", [8, 128, 256], BF16, isOutput=False)
    wk = nc.declare_dram_parameter("wk", [8, 128, 256], BF16, isOutput=False)
    wv = nc.declare_dram_parameter("wv", [8, 128, 260], BF16, isOutput=False)
    bqp = nc.declare_dram_parameter("bq", [128, 2], F32, isOutput=False)
    bkp = nc.declare_dram_parameter("bk", [128, 2], F32, isOutput=False)
    bvcp = nc.declare_dram_parameter("bvc", [1, 260], BF16, isOutput=False)
    wo = nc.declare_dram_parameter("wo", [128, 2, 1024], BF16, isOutput=False)
    maskp = nc.declare_dram_parameter("mask", [128, 128], BF16, isOutput=False)
    outp = nc.declare_dram_parameter("out", [N, 1024], BF16, isOutput=True)

    with tile.TileContext(nc) as tc:
        with tc.tile_pool(name="singles", bufs=1) as singles:
            xt_sb = singles.tile([128, 8, N], BF16)
            wq_sb = singles.tile([128, 8, 256], BF16)
            wk_sb = singles.tile([128, 8, 256], BF16)
            wv_sb = singles.tile([128, 8, 260], BF16)
            bq_sb = singles.tile([128, 2], F32)
            bk_sb = singles.tile([128, 2], F32)
            bvc_sb = singles.tile([1, 260], BF16)
            wo_sb = singles.tile([128, 2, 1024], BF16)
            mask_sb = singles.tile([128, 2, 128], BF16)
            ones_sb = singles.tile([1, 128], BF16)
            qT_sb = singles.tile([128, 2, N], BF16)
            kT_sb = singles.tile([128, 2, N], BF16)
            vc_sb = singles.tile([128, 16, 260], BF16)
            ctxn_sb = singles.tile([128, 2, N], BF16)

            def _dma_main():
                # weights on the SWDGE path, activations on HWDGE — parallel
                # issue queues; one large strided DMA per tensor.  These
                # tiles' last readers all sit in phases 0-2, so when emitted
                # mid-body as a prefetch the transfers start well before the
                # next body needs them.
                nc.gpsimd.dma_start(
                    out=wq_sb[:, :, :], in_=wq[:, :, :].rearrange("k p n -> p k n")
                )
                nc.gpsimd.dma_start(
                    out=wk_sb[:, :, :], in_=wk[:, :, :].rearrange("k p n -> p k n")
                )
                nc.gpsimd.dma_start(
                    out=wv_sb[:, :, :], in_=wv[:, :, :].rearrange("k p n -> p k n")
                )
                nc.gpsimd.dma_start(out=bq_sb[:, :], in_=bqp[:, :])
                nc.gpsimd.dma_start(out=bk_sb[:, :], in_=bkp[:, :])
                nc.gpsimd.dma_start(out=bvc_sb[:, :], in_=bvcp[:, :])
                nc.sync.dma_start(out=xt_sb[:, 0, :], in_=xT[0])
                nc.sync.dma_start(out=xt_sb[:, 1, :], in_=xT[1])
                for half in range(3):
                    k0 = 2 * half + 2
                    nc.sync.dma_start(
                        out=xt_sb[:, k0 : k0 + 2, :],
                        in_=xT[k0 : k0 + 2, :, :].rearrange("k p n -> p k n"),
                    )

            def _dma_tail():
                # mask is read by phase-3 mask-muls and wo by the tail final
                # units, so their WAR edges only clear at body end — emit
                # these prefetches after the tail to avoid stalling the
                # gpsimd queue (and the norm broadcasts behind it).
                nc.gpsimd.dma_start(out=mask_sb[:, 0, :], in_=maskp[:, :])
                nc.gpsimd.dma_start(out=mask_sb[:, 1, :], in_=maskp[:, :])
                nc.gpsimd.dma_start(out=wo_sb[:, :, :], in_=wo[:, :, :])

            def _qk_unit(scp, w_sb, b_sb, o_sb, c, I):
                ps = scp.tile([128, 1024], F32, tag="sc", name="qkps")
                for kc in range(8):
                    nc.tensor.matmul(
                        ps[:, :512],
                        lhsT=w_sb[:, kc, 128 * c : 128 * (c + 1)],
                        rhs=xt_sb[:, kc, 512 * I : 512 * (I + 1)],
                        start=(kc == 0),
                        stop=(kc == 7),
                    )
                nc.vector.tensor_scalar_add(
                    o_sb[:, c, 512 * I : 512 * (I + 1)],
                    ps[:, :512],
                    b_sb[:, c : c + 1],
                )

            def _v_unit(scp, J):
                ps = scp.tile([128, 1024], F32, tag="sc", name="vps")
                for kc in range(8):
                    nc.tensor.matmul(
                        ps[:, :260],
                        lhsT=xt_sb[:, kc, 128 * J : 128 * (J + 1)],
                        rhs=wv_sb[:, kc, :],
                        start=(kc == 0),
                        stop=False,
                    )
                nc.tensor.matmul(
                    ps[:, :260],
                    lhsT=ones_sb[:, :],
                    rhs=bvc_sb[:, :],
                    start=False,
                    stop=True,
                )
                nc.vector.tensor_copy(out=vc_sb[:, J, :], in_=ps[:, :260])

            def _norm_pair(znp, I, ctx_t, pair):
                """Normalize one pair's two heads for phase I."""
                heads = [(pair, hh) for hh in range(2)]
                if variant == "nopb":
                    for p, hh in heads:
                        h = 2 * p + hh
                        c, po = h // 2, 64 * (h % 2)
                        nc.vector.tensor_copy(
                            out=ctxn_sb[po : po + 64, c, 512 * I : 512 * (I + 1)],
                            in_=ctx_t[(p, hh)][0:64, :],
                        )
                    return
                # per-head chains: custom-DVE recip must read base partition
                # 0, and partition_broadcast only broadcasts partition 0 —
                # so each head gets its own partition-0 staging tiles
                for idx, (p, hh) in enumerate(heads):
                    h = 2 * p + hh
                    c, po = h // 2, 64 * (h % 2)
                    zs = znp.tile([1, 512], F32, tag=f"zs{idx % 2}", name="zs")
                    nc.scalar.copy(out=zs[:, :], in_=ctx_t[(p, hh)][64:65, :])
                    zr = znp.tile([1, 512], F32, tag=f"zr{idx % 2}", name="zr")
                    nc.vector.reciprocal_approx_fast(zr[:, :], zs[:, :])
                    zb = znp.tile([64, 512], F32, tag=f"zb{idx % 2}", name="zb")
                    nc.gpsimd.partition_broadcast(zb[:, :], zr[:, :], channels=64)
                    nc.vector.tensor_mul(
                        ctxn_sb[po : po + 64, c, 512 * I : 512 * (I + 1)],
                        ctx_t[(p, hh)][0:64, :],
                        zb[:, :],
                    )

            def _final_unit(scp, osb, t, oc):
                ps = scp.tile([128, 1024], F32, tag="sc", name="fps")
                for a in range(2):
                    nc.tensor.matmul(
                        ps[:, :512],
                        lhsT=ctxn_sb[:, a, 128 * t : 128 * (t + 1)],
                        rhs=wo_sb[:, a, 512 * oc : 512 * (oc + 1)],
                        start=(a == 0),
                        stop=(a == 1),
                    )
                ot = osb.tile([128, 512], BF16, tag="o", name="ot")
                if (t + oc) % 2 == 0:
                    nc.vector.tensor_copy(out=ot[:, :], in_=ps[:, :512])
                else:
                    nc.scalar.copy(out=ot[:, :], in_=ps[:, :512])
                nc.sync.dma_start(
                    out=outp[
                        128 * t : 128 * (t + 1),
                        512 * oc : 512 * (oc + 1),
                    ],
                    in_=ot[:, :],
                )

            def _iter(prefetch=False):
                with tc.tile_pool(name="sc_ps", bufs=2, space="PSUM") as scp, \
                     tc.tile_pool(name="ctx_ps", bufs=1, space="PSUM") as ctxp, \
                     tc.tile_pool(name="pt", bufs=6) as ptp, \
                     tc.tile_pool(name="zn", bufs=2) as znp, \
                     tc.tile_pool(name="osb", bufs=4) as osb:

                    nc.vector.memset(ones_sb[:, :], 1.0)

                    # ---- ramp: minimum inputs for phase 0 of both pairs ----
                    _qk_unit(scp, wq_sb, bq_sb, qT_sb, 0, 0)
                    _qk_unit(scp, wk_sb, bk_sb, kT_sb, 0, 0)
                    _qk_unit(scp, wq_sb, bq_sb, qT_sb, 1, 0)
                    _qk_unit(scp, wk_sb, bk_sb, kT_sb, 1, 0)
                    for J in range(4):
                        _v_unit(scp, J)

                    def qku(w_sb, b_sb, o_sb, c, I):
                        return lambda: _qk_unit(scp, w_sb, b_sb, o_sb, c, I)

                    def vu(J):
                        return lambda: _v_unit(scp, J)

                    def fin(t, oc):
                        return lambda: _final_unit(scp, osb, t, oc)

                    def qk4(I):
                        return [
                            qku(wq_sb, bq_sb, qT_sb, 0, I),
                            qku(wk_sb, bk_sb, kT_sb, 0, I),
                            qku(wq_sb, bq_sb, qT_sb, 1, I),
                            qku(wk_sb, bk_sb, kT_sb, 1, I),
                        ]

                    def proj_fill(I):
                        # interleave this I-window's v units and qk units
                        vs = [vu(J) for J in range(4 * I, 4 * I + 4)]
                        qs = qk4(I)
                        out = []
                        for a, b in zip(vs, qs):
                            out += [a, b]
                        return out

                    def fin8(I):
                        return [
                            fin(t, oc)
                            for t in range(4 * I, 4 * I + 4)
                            for oc in range(2)
                        ]

                    fillers = {
                        0: proj_fill(1),
                        1: proj_fill(2) + fin8(0),
                        2: proj_fill(3) + fin8(1),
                        # two empty slots so no final unit is emitted before
                        # the J0/J1-deferred norms whose ctxn it reads
                        3: [None, None] + fin8(2),
                    }

                    # ---- merged attention: pairs interleaved per (phase, J)
                    pend = {0: [], 1: []}
                    ctx_t = {}

                    def _emit_ctx(p, phase, J, pt, L, gs0):
                        lo = gs0 - 512 * phase
                        for hh in range(2):
                            h = 2 * p + hh
                            nc.tensor.matmul(
                                ctx_t[(p, hh)][:, lo : lo + L],
                                lhsT=vc_sb[:, J, 65 * h : 65 * h + 65],
                                rhs=pt[:, hh, :L],
                                start=(J == 0),
                                stop=(J == 4 * phase + 3),
                            )

                    def _flush(p, keep=0):
                        # ctx matmuls ride `keep` steps behind their exp so
                        # the exp/mask chain (and, at phase starts, the
                        # previous phase's norm reads of the ctx slot) never
                        # head-block the in-order PE queue
                        while len(pend[p]) > keep:
                            _emit_ctx(*pend[p].pop(0))

                    def _step(p, phase, J):
                        c = p
                        i0 = 512 * phase
                        gs0 = max(i0, 128 * J)
                        L = i0 + 512 - gs0
                        ps = scp.tile([128, 2, 512], F32, tag="sc", name="scps")
                        pt = ptp.tile([128, 2, 512], BF16, tag="pt", name="pt")
                        for hh in range(2):
                            po = 64 * hh
                            nc.tensor.matmul(
                                ps[:, hh, :L],
                                lhsT=kT_sb[
                                    po : po + 64, c, 128 * J : 128 * (J + 1)
                                ],
                                rhs=qT_sb[po : po + 64, c, gs0 : gs0 + L],
                                start=True,
                                stop=True,
                            )
                        _flush(p, keep=0)
                        nc.scalar.activation(
                            pt[:, :, :L], ps[:, :, :L], AF.Exp, scale=0.125
                        )
                        if J >= 4 * phase and variant != "nomask":
                            nc.vector.tensor_mul(
                                pt[:, :, :128], pt[:, :, :128], mask_sb[:, :, :]
                            )
                        pend[p].append((p, phase, J, pt, L, gs0))

                    deferred_norm = [None, None]

                    for phase in range(4):
                        for p in range(2):
                            for hh in range(2):
                                ctx_t[(p, hh)] = ctxp.tile(
                                    [65, 512],
                                    F32,
                                    name=f"ctx{p}{hh}",
                                    tag=f"ctx{p}{hh}",
                                )
                        fl = fillers[phase]
                        for J in range(4 * phase + 4):
                            _step(0, phase, J)
                            if J < 2 and deferred_norm[J] is not None:
                                # previous phase's norms land here (pair 0's
                                # heads at J=0, pair 1's at J=1, before that
                                # pair's first ctx flush of this phase):
                                # behind the phase-end fillers' DVE
                                # consumers, and split so the zs burst on
                                # ACT never delays the next exps by more
                                # than one chain-pair — but BEFORE any
                                # filler that might read ctxn
                                deferred_norm[J]()
                                deferred_norm[J] = None
                            if fl:
                                f = fl.pop(0)
                                if f is not None:
                                    f()
                            _step(1, phase, J)
                            if fl:
                                f = fl.pop(0)
                                if f is not None:
                                    f()
                        for p in range(2):
                            _flush(p)
                        while fl:
                            f = fl.pop(0)
                            if f is not None:
                                f()
                        snap = dict(ctx_t)
                        deferred_norm[0] = (
                            lambda ph=phase, ct=snap: _norm_pair(znp, ph, ct, 0)
                        )
                        deferred_norm[1] = (
                            lambda ph=phase, ct=snap: _norm_pair(znp, ph, ct, 1)
                        )
                        if phase == 2 and prefetch:
                            # next body's inputs: transfers start as soon as
                            # this body's last readers finish (mid-phase 2)
                            _dma_main()

                    deferred_norm[0]()
                    deferred_norm[1]()
                    for t_oc in fin8(3):
                        t_oc()
                    if prefetch:
                        _dma_tail()

            if loop == 1:
                _dma_main()
                _dma_tail()
                _iter()
            else:
                assert loop % unroll == 0
                _dma_main()
                _dma_tail()
                with tc.For_i(0, loop // unroll, 1):
                    for _ in range(unroll):
                        _iter(prefetch=True)

    nc.compile()
    return nc


class _Runner:
    """Jitted PJRT executor for the SPMD program (built once per process)."""

    def __init__(self, nc):
        import jax
        from jax.experimental.shard_map import shard_map
        from jax.sharding import Mesh, NamedSharding, PartitionSpec

        from concourse.bass2jax import (
            _bass_exec_p,
            install_neuronx_cc_hook,
            partition_id_tensor,
        )

        install_neuronx_cc_hook()
        self.nc = nc
        self.jax = jax

        in_names, out_names, out_avals = [], [], []
        partition_name = (
            nc.partition_id_tensor.name if nc.partition_id_tensor else None
        )
        for alloc in nc.m.functions[0].allocations:
            if not isinstance(alloc, mybir.MemoryLocationSet):
                continue
            name = alloc.memorylocations[0].name
            if alloc.kind == "ExternalInput":
                if name != partition_name:
                    in_names.append(name)
            elif alloc.kind == "ExternalOutput":
                out_names.append(name)
                out_avals.append(
                    jax.core.ShapedArray(
                        tuple(alloc.tensor_shape), mybir.dt.np(alloc.dtype)
                    )
                )
        self.in_names = list(in_names)
        self.out_names = out_names
        self.out_avals = out_avals
        n_params = len(in_names)
        n_outs = len(out_names)
        all_names = in_names + out_names
        if partition_name is not None:
            all_names = all_names + [partition_name]

        def _body(*args):
            operands = list(args)
            if partition_name is not None:
                operands.append(partition_id_tensor())
            return tuple(
                _bass_exec_p.bind(
                    *operands,
                    out_avals=tuple(out_avals),
                    in_names=tuple(all_names),
                    out_names=tuple(out_names),
                    lowering_input_output_aliases=(),
                    sim_require_finite=True,
                    sim_require_nnan=True,
                    nc=nc,
                )
            )

        devices = jax.devices()[:NC]
        self.mesh = Mesh(np.asarray(devices), ("core",))
        in_specs = (PartitionSpec("core"),) * (n_params + n_outs)
        out_specs = (PartitionSpec("core"),) * n_outs
        self.fn = jax.jit(
            shard_map(
                _body,
                mesh=self.mesh,
                in_specs=in_specs,
                out_specs=out_specs,
                check_rep=False,
            ),
            keep_unused=True,
        )
        self.sharding = NamedSharding(self.mesh, PartitionSpec("core"))

    def prep(self, in_maps):
        """Concatenate per-core inputs along axis 0 and device_put."""
        arrs = []
        for name in self.in_names:
            arrs.append(np.concatenate([m[name] for m in in_maps], axis=0))
        for av in self.out_avals:
            arrs.append(np.zeros((NC * av.shape[0], *av.shape[1:]), av.dtype))
        return [self.jax.device_put(a, self.sharding) for a in arrs]

    def run(self, dev_args):
        out = self.fn(*dev_args)
        self.jax.block_until_ready(out)
        return out

    def run_async(self, dev_args):
        return self.fn(*dev_args)

    def unpack(self, out):
        res = []
        for c in range(NC):
            res.append(
                {
                    name: np.asarray(out[i]).reshape(NC, *self.out_avals[i].shape)[c]
                    for i, name in enumerate(self.out_names)
                }
            )
        return res


_RUNNER = None


def _get_runner():
    global _RUNNER
    if _RUNNER is None:
        _RUNNER = _Runner(build_nc())
    return _RUNNER


def make_in_maps(x, Wq, bq, Wk, bk, Wv, bv, Wo, bo):
    bf = ml_dtypes.bfloat16
    f32 = np.float32
    x = np.asarray(x, f32)
    Wq, Wk, Wv = (np.asarray(a, f32) for a in (Wq, Wk, Wv))
    bv = np.asarray(bv, f32)
    mask = np.ascontiguousarray(np.triu(np.ones((128, 128), f32))).astype(bf)
    in_maps = []
    for core in range(NC):
        b, g = core // 4, core % 4
        sl = slice(256 * g, 256 * (g + 1))
        wv_cat = np.zeros((D, 260), f32)
        bv_cat = np.zeros((1, 260), f32)
        for h in range(4):
            col = 256 * g + 64 * h
            wv_cat[:, 65 * h : 65 * h + 64] = Wv[:, col : col + 64]
            bv_cat[0, 65 * h : 65 * h + 64] = bv[col : col + 64]
            bv_cat[0, 65 * h + 64] = 1.0
        in_maps.append(
            {
                "xT": np.ascontiguousarray(x[b].T).reshape(8, 128, N).astype(bf),
                "wq": np.ascontiguousarray(Wq[:, sl]).reshape(8, 128, 256).astype(bf),
                "wk": np.ascontiguousarray(Wk[:, sl]).reshape(8, 128, 256).astype(bf),
                "wv": wv_cat.reshape(8, 128, 260).astype(bf),
                "bq": np.ascontiguousarray(np.asarray(bq, f32)[sl].reshape(2, 128).T),
                "bk": np.ascontiguousarray(np.asarray(bk, f32)[sl].reshape(2, 128).T),
                "bvc": bv_cat.astype(bf),
                "wo": np.ascontiguousarray(
                    np.asarray(Wo, f32)[sl].reshape(2, 128, 1024).transpose(1, 0, 2)
                ).astype(bf),
                "mask": mask,
            }
        )
    return in_maps


def combine(results, bo):
    out = np.zeros((2, N, D), np.float32)
    for core in range(NC):
        out[core // 4] += results[core]["out"].astype(np.float32)
    out += np.asarray(bo, np.float32)[None, None, :]
    return out


def kernel(x, Wq, bq, Wk, bk, Wv, bv, Wo, bo):
    runner = _get_runner()
    in_maps = make_in_maps(x, Wq, bq, Wk, bk, Wv, bv, Wo, bo)
    dev_args = runner.prep(in_maps)
    results = runner.unpack(runner.run(dev_args))
    return combine(results, bo)
